# revision 55
# baseline (speedup 1.0000x reference)
"""Multi-head dot-attention kernel for Trainium2, 8-core batch-parallel.

out[b] = concat_h( softmax((x_b WQ_h)(x_b WK_h)^T / sqrt(E)) (x_b WV_h) )

Sharding: batch b -> core b (8 batches, 8 cores). Each core runs the same
program on its own batch slice; weights are broadcast.

Per-core pipeline (all shapes [partition, free]):
  1. x_b [S,D] DMA'd per s-chunk, PE-transposed to xT [D,S] (stationary
     operand for all projections).
  2. V projection batched over all heads: V1[t, h, 0:E] = x_b @ WV, with a
     ones column at V1[t, h, E] so the O^T matmul also produces softmax
     denominators.
  3. Per head pair (2E = 128): Q^T/K^T computed directly in [2E, S] layout
     with a single M=128 weight covering both heads (lhsT = [d, 2E]).
  4. S^T[t,s] = K^T.T @ Q^T per head; the pair's two matmuls use disjoint
     contraction partitions (0:64 / 64:128) and share one psum tile, so the
     PE co-executes them (row-strip packing). One exp() per tile covers both
     heads straight out of PSUM with the 1/sqrt(E) scale folded in; no max
     subtraction (|scores| <= ~12 for these inputs, exp is safe in fp32).
  5. O'^T[e1, s] = [V_h | 1]^T @ expS^T accumulated over t-chunks; row E is
     the softmax denominator. PE-transpose back to [s, e1], multiply by the
     reciprocal of column E, DMA out.
The per-pair phases are software-pipelined: pair p's ACT-bound score phase
is filled with pair p+1's projections, pair p-1's O matmuls/finalize, and
the V projection.
"""

import math
import os

import numpy as np

import concourse.bass as bass
import concourse.mybir as mybir
from concourse import bacc
from concourse.bass import ds, ts
from concourse.masks import make_identity
from concourse.tile import TileContext

P = 128
F32 = mybir.dt.float32
BF16 = mybir.dt.bfloat16
F32R = mybir.dt.float32r

N_CORES = 8
FULL = dict(S=1024, D=1024, H=16, E=64)


def build_nc(S=1024, D=1024, H=16, E=64, st_dt=BF16, es_dt=None):
    """Build the single-core Bass program (SPMD across cores).

    st_dt: SBUF storage dtype for matmul operands: BF16, F32R, or F32.
    es_dt: storage dtype for expS^T and V1 (defaults to st_dt); BF16 with
        st_dt=F32R gives the hybrid variant (f32r scores, bf16 A@V).
    """
    if es_dt is None:
        es_dt = st_dt
    assert E == 64 and P == 2 * E
    SC = S // P            # s- (and t-) chunks of 128
    DC = D // P            # d-chunks of 128
    HE = H * E
    NW = min(512, HE)      # he tile width for the V projection
    NHE = HE // NW
    HPW = NW // E          # heads per he tile
    S2 = min(512, S)       # matmul moving width
    NS2 = S // S2
    E1 = E + 1
    scale = 1.0 / math.sqrt(E)
    assert S % S2 == 0 and H % 2 == 0

    in_dt = F32R if st_dt == F32R else F32
    nc = bacc.Bacc("TRN2", target_bir_lowering=False)
    x = nc.dram_tensor("x", [S, D], in_dt, kind="ExternalInput")
    WQ = nc.dram_tensor("WQ", [H, D, E], in_dt, kind="ExternalInput")
    WK = nc.dram_tensor("WK", [H, D, E], in_dt, kind="ExternalInput")
    WV = nc.dram_tensor("WV", [H, D, E], in_dt, kind="ExternalInput")
    out = nc.dram_tensor("out", [H, SC, P, E], F32, kind="ExternalOutput")

    with TileContext(nc) as tc:
        with (
            tc.tile_pool(name="const", bufs=1) as const,
            tc.tile_pool(name="persist", bufs=1) as persist,
            tc.tile_pool(name="stage", bufs=2) as stage,
            tc.tile_pool(name="wpool", bufs=2) as wpool,
            tc.tile_pool(name="qk", bufs=2) as qk,
            tc.tile_pool(name="es", bufs=2 if es_dt == BF16 else 1) as es_pool,
            tc.tile_pool(name="ot", bufs=3) as ot_pool,
            tc.tile_pool(name="osb", bufs=3) as osb,
            tc.tile_pool(name="small", bufs=2) as small,
            tc.tile_pool(name="ps_pj", bufs=2, space="PSUM") as ps_pj,
            tc.tile_pool(name="ps_s", bufs=2, space="PSUM") as ps_s,
            tc.tile_pool(name="ps_po", bufs=2, space="PSUM") as ps_po,
        ):
            id_f32 = const.tile([P, P], F32, tag="idf")
            make_identity(nc, id_f32[:])
            if st_dt != F32:
                id_mm = const.tile([P, P], st_dt, tag="idb")
                nc.vector.tensor_copy(out=id_mm[:], in_=id_f32[:])
            else:
                id_mm = id_f32

            def load_wpair(pr):
                # W slices for pair `pr`: [d, 2E] per d-chunk
                h0 = 2 * pr
                wp = {}
                for name, W in (("q", WQ), ("k", WK)):
                    wp[name] = wpool.tile(
                        [P, DC, 2 * E], st_dt, tag=f"w{name}p", name=f"w{name}p{pr}"
                    )
                    wst = (
                        wp[name]
                        if st_dt != BF16
                        else stage.tile(
                            [P, DC, 2 * E], F32, tag="wst2", name=f"wst2{pr}_{name}"
                        )
                    )
                    for hh_ in range(2):
                        nc.sync.dma_start(
                            out=wst[:, :, ds(hh_ * E, E)],
                            in_=W[h0 + hh_].rearrange("(dc p) e -> p dc e", p=P),
                        )
                    if st_dt == BF16:
                        nc.vector.tensor_copy(out=wp[name][:], in_=wst[:])
                return wp

            # prefetch pair 0's weights so its projections can start the
            # moment x^T is ready (the strided W DMAs are slow; queued after
            # the x loads they leave the PE idle for ~6us at startup)
            wps = {0: load_wpair(0)}

            # ---- x transpose: xt[dc][s2] holds x^T[d-chunk, s-half] ----
            # In bf16 mode, cast the x chunk first so the PE transposes run
            # at 1 cyc/row instead of 2 (and psum traffic halves).
            xt = [
                [
                    persist.tile(
                        [P, S2], st_dt, tag=f"xt{dc}_{s2}", name=f"xt{dc}_{s2}"
                    )
                    for s2 in range(NS2)
                ]
                for dc in range(DC)
            ]
            for sc in range(SC):
                xs = stage.tile([P, D], in_dt, tag="xs")
                nc.sync.dma_start(out=xs[:], in_=x[ts(sc, P), :])
                if st_dt == BF16:
                    xb = stage.tile([P, D], BF16, tag="xb")
                    nc.vector.tensor_copy(out=xb[:], in_=xs[:])
                else:
                    xb = xs  # already in_dt == st_dt for F32R/F32
                for dc in range(DC):
                    tp = ps_po.tile([P, P], st_dt, tag="po")
                    nc.tensor.transpose(tp[:], xb[:, ts(dc, P)], id_mm[:])
                    sh = SC // NS2  # s-chunks per half
                    nc.vector.tensor_copy(
                        out=xt[dc][sc // sh][:, ts(sc % sh, P)], in_=tp[:]
                    )

            # ---- V projection + ones column (emitted per head-group inside
            # the pair loop so pair 0's scores/exp start sooner) ----
            V1 = persist.tile([P, SC, H, E1], es_dt, tag="V1")
            if es_dt == F32R:
                # memset can't emit f32r; route the ones through a casting copy
                ones_f = const.tile([P, SC * H], F32, tag="ones")
                nc.vector.memset(ones_f[:], 1.0)
                nc.vector.tensor_copy(
                    out=V1[:, :, :, E:E1].rearrange("p a b c -> p (a b c)"),
                    in_=ones_f[:],
                )
            else:
                nc.vector.memset(V1[:, :, :, E:E1], 1.0)

            def vproj_jobs(he2):
                # DMA the WV slice now; return one per-t-chunk matmul job
                wvb = wpool.tile(
                    [P, DC, NW], st_dt, tag="wvb", name=f"wvb{he2}",
                    bufs=1 if st_dt != BF16 else 2,
                )
                wst = (
                    wvb
                    if st_dt != BF16
                    else stage.tile(
                        [P, DC, NW], F32, tag="wst", name=f"wvst{he2}", bufs=1
                    )
                )
                for hh_ in range(HPW):
                    nc.sync.dma_start(
                        out=wst[:, :, ds(hh_ * E, E)],
                        in_=WV[he2 * HPW + hh_].rearrange("(dc p) e -> p dc e", p=P),
                    )
                if st_dt == BF16:
                    nc.vector.tensor_copy(out=wvb[:], in_=wst[:])

                def job(tcj):
                    pv = ps_pj.tile([P, NW], F32, tag="pj", name=f"pv{he2}_{tcj}")
                    sh = SC // NS2
                    for dc in range(DC):
                        nc.tensor.matmul(
                            pv[:],
                            xt[dc][tcj // sh][:, ts(tcj % sh, P)],
                            wvb[:, dc, :],
                            start=(dc == 0),
                            stop=(dc == DC - 1),
                        )
                    nc.vector.tensor_copy(
                        out=V1[:, tcj, ds(he2 * HPW, HPW), 0:E],
                        in_=pv[:].rearrange("p (h e) -> p h e", e=E),
                    )

                return [lambda tcj=tcj: job(tcj) for tcj in range(SC)]

            # ---- per-pair phases, software-pipelined ----
            # Pair p's S+exp phase is ACT-bound (the PE is ~20% busy in it),
            # while projections and the O phase are PE-dense with the ACT
            # idle. Interleave: during pair p's S phase, emit pair p+1's
            # Q/K projections, pair p-1's O matmuls + finalize, and the V
            # projection (pairs 0..NHE-1) as fill work between score tiles.

            def emit_qkproj(pr, wp, qt2, kt2):
                # one job per (tensor, s-half): an 8-matmul chain + copy
                jobs = []
                for name, dst in (("q", qt2), ("k", kt2)):
                    for s2 in range(NS2):
                        def job(name=name, dst=dst, s2=s2):
                            pq = ps_pj.tile(
                                [P, S2], F32, tag="pj", name=f"pq{pr}_{name}_{s2}"
                            )
                            # lhsT [d, 2E=128] covers BOTH heads
                            for dc in range(DC):
                                nc.tensor.matmul(
                                    pq[:],
                                    wp[name][:, dc, :],
                                    xt[dc][s2][:],
                                    start=(dc == 0),
                                    stop=(dc == DC - 1),
                                )
                            nc.vector.tensor_copy(
                                out=dst[:, ds(s2 * S2, S2)], in_=pq[:]
                            )
                        jobs.append(job)
                return jobs

            def o_jobs(pr, es2):
                # O'^T = [V|1]^T @ expS^T per (head, s-half): accumulation
                # chain + evacuate, then transpose/normalize; DMA per head.
                jobs = []
                sh = SC // NS2
                state = {}
                for hi in range(2):
                    hh = 2 * pr + hi
                    def alloc(hh=hh):
                        state[hh] = (
                            ot_pool.tile([E1, S], F32, tag="ot", name=f"ot{hh}"),
                            osb.tile([P, SC, E], F32, tag="ob", name=f"ob{hh}"),
                        )
                    for s2 in range(NS2):
                        def chain(hi=hi, hh=hh, s2=s2):
                            if hh not in state:
                                alloc(hh)
                            ot_sb, _ = state[hh]
                            po = ps_po.tile(
                                [E1, S2], F32, tag="po", name=f"po{hh}_{s2}"
                            )
                            for tcj in range(SC):
                                nc.tensor.matmul(
                                    po[:],
                                    V1[:, tcj, hh, :],
                                    es2[:, tcj, hi, ds(s2 * S2, S2)],
                                    start=(tcj == 0),
                                    stop=(tcj == SC - 1),
                                )
                            nc.vector.tensor_copy(
                                out=ot_sb[:, ds(s2 * S2, S2)], in_=po[:]
                            )
                        def fin(hh=hh, s2=s2):
                            ot_sb, ob = state[hh]
                            for sc in range(s2 * sh, (s2 + 1) * sh):
                                tp = ps_po.tile(
                                    [P, P], F32, tag="po", name=f"tp{hh}_{sc}"
                                )
                                nc.tensor.transpose(
                                    tp[:, 0:E1],
                                    ot_sb[:, ts(sc, P)],
                                    id_f32[0:E1, 0:E1],
                                )
                                rec = small.tile([P, 1], F32, tag="rec")
                                nc.vector.reciprocal(out=rec[:], in_=tp[:, E:E1])
                                # normalize on the scalar engine (idle during
                                # the O phase) to keep the finalize chain off
                                # the vector engine
                                nc.scalar.activation(
                                    out=ob[:, sc, :],
                                    in_=tp[:, 0:E],
                                    func=mybir.ActivationFunctionType.Copy,
                                    scale=rec[:],
                                )
                            if s2 == NS2 - 1:
                                nc.sync.dma_start(
                                    out=out[hh].rearrange("sc p e -> p sc e"),
                                    in_=ob[:],
                                )
                        jobs.append(chain)
                        jobs.append(fin)
                return jobs

            H2 = H // 2
            qts = {}
            pending_o = []
            for pr in range(H2):
                wp = wps.pop(pr)
                if pr + 1 < H2:
                    wps[pr + 1] = load_wpair(pr + 1)

                if pr == 0:
                    # pair 0's projections run serially at startup; do the
                    # s-half-0 chains of BOTH q and k first so the first
                    # score tiles only wait on half of x / W
                    qt2 = qk.tile([P, S], st_dt, tag="qt2", name="qt2_0")
                    kt2 = qk.tile([P, S], st_dt, tag="kt2", name="kt2_0")
                    qts[0] = (qt2, kt2)
                    jobs0 = emit_qkproj(0, wp, qt2, kt2)
                    for i in (0, NS2, 1, NS2 + 1) if NS2 > 1 else (0, 1):
                        jobs0[i]()
                qt2, kt2 = qts.pop(pr)

                # fill queue for this S phase
                fill = list(pending_o)
                pending_o = []
                if pr + 1 < H2:
                    nqt = qk.tile([P, S], st_dt, tag="qt2", name=f"qt2_{pr+1}")
                    nkt = qk.tile([P, S], st_dt, tag="kt2", name=f"kt2_{pr+1}")
                    qts[pr + 1] = (nqt, nkt)
                    fill += emit_qkproj(pr + 1, wps[pr + 1], nqt, nkt)
                if pr < NHE:
                    fill += vproj_jobs(pr)

                # scores + exp. Both heads' tiles for one (tcj, s2) share ONE
                # psum tile (adjacent banks): the matmuls use disjoint PE row
                # strips and issue back-to-back so the hardware co-executes
                # them; the single exp frees both banks simultaneously.
                es2 = es_pool.tile([P, SC, 2, S], es_dt, tag="es", name=f"es{pr}")

                # (tcj, s2) emission order: pair 0 starts with the tiles that
                # only need the s-half-0 projections; the last pair goes
                # s2-major so its own s2=0 O chains can run inside the phase.
                sh = SC // NS2
                if pr == H2 - 1 and NS2 > 1:
                    order = [(t, s) for s in range(NS2) for t in range(SC)]
                elif pr == 0 and NS2 > 1:
                    order = (
                        [(t, 0) for t in range(sh)]
                        + [(t, 1) for t in range(sh)]
                        + [(t, s) for t in range(sh, SC) for s in range(NS2)]
                    )
                else:
                    order = [(t, s) for t in range(SC) for s in range(NS2)]

                last_o = o_jobs(pr, es2) if pr == H2 - 1 else None
                if last_o is not None and NS2 > 1:
                    # s2=0 chains + finalizes of the last pair join the fill
                    # (they land in the second, s2=1 half of the phase)
                    fill = fill + [last_o[i] for i in (0, 1, 4, 5)]

                done = 0
                for k, (tcj, s2) in enumerate(order):
                    ps2 = ps_s.tile(
                        [P, 2, S2], F32, tag="s", name=f"ps{pr}_{tcj}_{s2}"
                    )
                    for hi in range(2):
                        nc.tensor.matmul(
                            ps2[:, hi, :],
                            kt2[ds(hi * E, E), ts(tcj, P)],
                            qt2[ds(hi * E, E), ds(s2 * S2, S2)],
                        )
                    nc.scalar.activation(
                        out=es2[:, tcj, :, ds(s2 * S2, S2)],
                        in_=ps2[:],
                        func=mybir.ActivationFunctionType.Exp,
                        scale=scale,
                    )
                    want = (k + 1) * len(fill) // len(order)
                    while done < want:
                        fill[done]()
                        done += 1

                if last_o is not None:
                    # both heads' s2=1 chains first, then their finalizes
                    rest = (2, 6, 3, 7) if NS2 > 1 else range(len(last_o))
                    for i in rest:
                        last_o[i]()
                    pending_o = []
                else:
                    pending_o = o_jobs(pr, es2)
            for job in pending_o:
                job()
    nc.finalize()
    return nc


_NC_CACHE = {}


def _get_nc(key=("bf16",)):
    if key not in _NC_CACHE:
        if key[0] == "bf16":
            _NC_CACHE[key] = build_nc(**FULL, st_dt=BF16)
        elif key[0] == "f32r":
            _NC_CACHE[key] = build_nc(**FULL, st_dt=F32R)
        elif key[0] == "hybrid":
            _NC_CACHE[key] = build_nc(**FULL, st_dt=F32R, es_dt=BF16)
        else:
            _NC_CACHE[key] = build_nc(**FULL, st_dt=F32)
    return _NC_CACHE[key]


# "bf16": fastest (~264 us, absmax-rel err ~1.2e-2)
# "hybrid": f32r scores + bf16 A@V (~285 us, ~2.7e-3)
# "f32r": max accuracy (~358 us, ~4.8e-4)
DEFAULT_VARIANT = os.environ.get("ATTN_VARIANT", "bf16")


def run_on_hw(x, WQ, WK, WV, variant=None, trace=False):
    from concourse.bass_utils import run_bass_kernel_spmd

    if variant is None:
        variant = DEFAULT_VARIANT

    nc = _get_nc((variant,))
    B = x.shape[0]
    assert B == N_CORES
    in_maps = [
        {
            "x": np.ascontiguousarray(x[b], dtype=np.float32),
            "WQ": np.ascontiguousarray(WQ, dtype=np.float32),
            "WK": np.ascontiguousarray(WK, dtype=np.float32),
            "WV": np.ascontiguousarray(WV, dtype=np.float32),
        }
        for b in range(B)
    ]
    res = run_bass_kernel_spmd(nc, in_maps, list(range(N_CORES)), trace=trace)
    outs = np.stack(
        [np.asarray(res.results[b]["out"]).reshape(-1) for b in range(B)], axis=0
    )
    return outs.astype(np.float32, copy=False), res


def kernel(x, WQ, WK, WV):
    outs, _ = run_on_hw(
        np.asarray(x), np.asarray(WQ), np.asarray(WK), np.asarray(WV)
    )
    return outs


# revision 56
# speedup vs baseline: 1.0564x; 1.0564x over previous
"""Multi-head dot-attention kernel for Trainium2, 8-core batch-parallel.

out[b] = concat_h( softmax((x_b WQ_h)(x_b WK_h)^T / sqrt(E)) (x_b WV_h) )

Sharding: batch b -> core b (8 batches, 8 cores). Each core runs the same
program on its own batch slice; weights are broadcast.

Per-core pipeline (all shapes [partition, free]):
  1. x_b [S,D] DMA'd per s-chunk, PE-transposed to xT [D,S] (stationary
     operand for all projections).
  2. V projection batched over all heads: V1[t, h, 0:E] = x_b @ WV, with a
     ones column at V1[t, h, E] so the O^T matmul also produces softmax
     denominators.
  3. Per head pair (2E = 128): Q^T/K^T computed directly in [2E, S] layout
     with a single M=128 weight covering both heads (lhsT = [d, 2E]).
  4. S^T[t,s] = K^T.T @ Q^T per head; the pair's two matmuls use disjoint
     contraction partitions (0:64 / 64:128) and share one psum tile, so the
     PE co-executes them (row-strip packing). One exp() per tile covers both
     heads straight out of PSUM with the 1/sqrt(E) scale folded in; no max
     subtraction (|scores| <= ~12 for these inputs, exp is safe in fp32).
  5. O'^T[e1, s] = [V_h | 1]^T @ expS^T accumulated over t-chunks; row E is
     the softmax denominator. PE-transpose back to [s, e1], multiply by the
     reciprocal of column E, DMA out.
The per-pair phases are software-pipelined: pair p's ACT-bound score phase
is filled with pair p+1's projections, pair p-1's O matmuls/finalize, and
the V projection.
"""

import math
import os

import numpy as np

import concourse.bass as bass
import concourse.mybir as mybir
from concourse import bacc
from concourse.bass import ds, ts
from concourse.masks import make_identity
from concourse.tile import TileContext

P = 128
F32 = mybir.dt.float32
BF16 = mybir.dt.bfloat16
F32R = mybir.dt.float32r

N_CORES = 8
FULL = dict(S=1024, D=1024, H=16, E=64)


def build_nc(S=1024, D=1024, H=16, E=64, st_dt=BF16, es_dt=None):
    """Build the single-core Bass program (SPMD across cores).

    st_dt: SBUF storage dtype for matmul operands: BF16, F32R, or F32.
    es_dt: storage dtype for expS^T and V1 (defaults to st_dt); BF16 with
        st_dt=F32R gives the hybrid variant (f32r scores, bf16 A@V).
    """
    if es_dt is None:
        es_dt = st_dt
    assert E == 64 and P == 2 * E
    SC = S // P            # s- (and t-) chunks of 128
    DC = D // P            # d-chunks of 128
    HE = H * E
    NW = min(512, HE)      # he tile width for the V projection
    NHE = HE // NW
    HPW = NW // E          # heads per he tile
    S2 = min(512, S)       # matmul moving width
    NS2 = S // S2
    E1 = E + 1
    scale = 1.0 / math.sqrt(E)
    assert S % S2 == 0 and H % 2 == 0

    in_dt = F32R if st_dt == F32R else F32
    nc = bacc.Bacc("TRN2", target_bir_lowering=False)
    x = nc.dram_tensor("x", [S, D], in_dt, kind="ExternalInput")
    WQ = nc.dram_tensor("WQ", [H, D, E], in_dt, kind="ExternalInput")
    WK = nc.dram_tensor("WK", [H, D, E], in_dt, kind="ExternalInput")
    WV = nc.dram_tensor("WV", [H, D, E], in_dt, kind="ExternalInput")
    out = nc.dram_tensor("out", [H, SC, P, E], F32, kind="ExternalOutput")

    with TileContext(nc) as tc:
        with (
            tc.tile_pool(name="const", bufs=1) as const,
            tc.tile_pool(name="persist", bufs=1) as persist,
            tc.tile_pool(name="stage", bufs=2) as stage,
            tc.tile_pool(name="wpool", bufs=2) as wpool,
            tc.tile_pool(name="qk", bufs=2) as qk,
            tc.tile_pool(name="es", bufs=2 if es_dt == BF16 else 1) as es_pool,
            tc.tile_pool(name="ot", bufs=3) as ot_pool,
            tc.tile_pool(name="osb", bufs=3) as osb,
            tc.tile_pool(name="small", bufs=2) as small,
            tc.tile_pool(name="ps_pj", bufs=2, space="PSUM") as ps_pj,
            tc.tile_pool(name="ps_s", bufs=2, space="PSUM") as ps_s,
            tc.tile_pool(name="ps_po", bufs=2, space="PSUM") as ps_po,
        ):
            id_f32 = const.tile([P, P], F32, tag="idf")
            make_identity(nc, id_f32[:])
            if st_dt != F32:
                id_mm = const.tile([P, P], st_dt, tag="idb")
                nc.vector.tensor_copy(out=id_mm[:], in_=id_f32[:])
            else:
                id_mm = id_f32

            def load_wpair(pr):
                # W slices for pair `pr`: [d, 2E] per d-chunk
                h0 = 2 * pr
                wp = {}
                for name, W in (("q", WQ), ("k", WK)):
                    wp[name] = wpool.tile(
                        [P, DC, 2 * E], st_dt, tag=f"w{name}p", name=f"w{name}p{pr}"
                    )
                    wst = (
                        wp[name]
                        if st_dt != BF16
                        else stage.tile(
                            [P, DC, 2 * E], F32, tag="wst2", name=f"wst2{pr}_{name}"
                        )
                    )
                    for hh_ in range(2):
                        nc.sync.dma_start(
                            out=wst[:, :, ds(hh_ * E, E)],
                            in_=W[h0 + hh_].rearrange("(dc p) e -> p dc e", p=P),
                        )
                    if st_dt == BF16:
                        nc.vector.tensor_copy(out=wp[name][:], in_=wst[:])
                return wp

            # prefetch pair 0's weights so its projections can start the
            # moment x^T is ready (the strided W DMAs are slow; queued after
            # the x loads they leave the PE idle for ~6us at startup)
            wps = {0: load_wpair(0)}

            # ---- x transpose: xt[dc][s2] holds x^T[d-chunk, s-half] ----
            # In bf16 mode, cast the x chunk first so the PE transposes run
            # at 1 cyc/row instead of 2 (and psum traffic halves).
            xt = [
                [
                    persist.tile(
                        [P, S2], st_dt, tag=f"xt{dc}_{s2}", name=f"xt{dc}_{s2}"
                    )
                    for s2 in range(NS2)
                ]
                for dc in range(DC)
            ]
            for sc in range(SC):
                xs = stage.tile([P, D], in_dt, tag="xs")
                nc.sync.dma_start(out=xs[:], in_=x[ts(sc, P), :])
                if st_dt == BF16:
                    xb = stage.tile([P, D], BF16, tag="xb")
                    nc.vector.tensor_copy(out=xb[:], in_=xs[:])
                else:
                    xb = xs  # already in_dt == st_dt for F32R/F32
                for dc in range(DC):
                    tp = ps_po.tile([P, P], st_dt, tag="po")
                    nc.tensor.transpose(tp[:], xb[:, ts(dc, P)], id_mm[:])
                    sh = SC // NS2  # s-chunks per half
                    nc.vector.tensor_copy(
                        out=xt[dc][sc // sh][:, ts(sc % sh, P)], in_=tp[:]
                    )

            # ---- V projection + ones column (emitted per head-group inside
            # the pair loop so pair 0's scores/exp start sooner) ----
            V1 = persist.tile([P, SC, H, E1], es_dt, tag="V1")
            if es_dt == F32R:
                # memset can't emit f32r; route the ones through a casting copy
                ones_f = const.tile([P, SC * H], F32, tag="ones")
                nc.vector.memset(ones_f[:], 1.0)
                nc.vector.tensor_copy(
                    out=V1[:, :, :, E:E1].rearrange("p a b c -> p (a b c)"),
                    in_=ones_f[:],
                )
            else:
                nc.vector.memset(V1[:, :, :, E:E1], 1.0)

            def vproj_jobs(he2):
                # DMA the WV slice now; return one per-t-chunk matmul job
                wvb = wpool.tile(
                    [P, DC, NW], st_dt, tag="wvb", name=f"wvb{he2}",
                    bufs=1 if st_dt != BF16 else 2,
                )
                wst = (
                    wvb
                    if st_dt != BF16
                    else stage.tile(
                        [P, DC, NW], F32, tag="wst", name=f"wvst{he2}", bufs=1
                    )
                )
                for hh_ in range(HPW):
                    nc.sync.dma_start(
                        out=wst[:, :, ds(hh_ * E, E)],
                        in_=WV[he2 * HPW + hh_].rearrange("(dc p) e -> p dc e", p=P),
                    )
                if st_dt == BF16:
                    nc.vector.tensor_copy(out=wvb[:], in_=wst[:])

                def job(tcj):
                    pv = ps_pj.tile([P, NW], F32, tag="pj", name=f"pv{he2}_{tcj}")
                    sh = SC // NS2
                    for dc in range(DC):
                        nc.tensor.matmul(
                            pv[:],
                            xt[dc][tcj // sh][:, ts(tcj % sh, P)],
                            wvb[:, dc, :],
                            start=(dc == 0),
                            stop=(dc == DC - 1),
                        )
                    nc.vector.tensor_copy(
                        out=V1[:, tcj, ds(he2 * HPW, HPW), 0:E],
                        in_=pv[:].rearrange("p (h e) -> p h e", e=E),
                    )

                return [lambda tcj=tcj: job(tcj) for tcj in range(SC)]

            # ---- per-pair phases, software-pipelined ----
            # Pair p's S+exp phase is ACT-bound (the PE is ~20% busy in it),
            # while projections and the O phase are PE-dense with the ACT
            # idle. Interleave: during pair p's S phase, emit pair p+1's
            # Q/K projections, pair p-1's O matmuls + finalize, and the V
            # projection (pairs 0..NHE-1) as fill work between score tiles.

            def emit_qkproj(pr, wp, qt2, kt2):
                # one job per (tensor, s-half): an 8-matmul chain + copy
                jobs = []
                for name, dst in (("q", qt2), ("k", kt2)):
                    for s2 in range(NS2):
                        def job(name=name, dst=dst, s2=s2):
                            pq = ps_pj.tile(
                                [P, S2], F32, tag="pj", name=f"pq{pr}_{name}_{s2}"
                            )
                            # lhsT [d, 2E=128] covers BOTH heads
                            for dc in range(DC):
                                nc.tensor.matmul(
                                    pq[:],
                                    wp[name][:, dc, :],
                                    xt[dc][s2][:],
                                    start=(dc == 0),
                                    stop=(dc == DC - 1),
                                )
                            nc.vector.tensor_copy(
                                out=dst[:, ds(s2 * S2, S2)], in_=pq[:]
                            )
                        jobs.append(job)
                return jobs

            def o_jobs(pr, es2):
                # O'^T = [V|1]^T @ expS^T per (head, s-half): accumulation
                # chain + evacuate, then transpose/normalize; DMA per head.
                jobs = []
                sh = SC // NS2
                state = {}
                for hi in range(2):
                    hh = 2 * pr + hi
                    def alloc(hh=hh):
                        state[hh] = (
                            ot_pool.tile([E1, S], F32, tag="ot", name=f"ot{hh}"),
                            osb.tile([P, SC, E], F32, tag="ob", name=f"ob{hh}"),
                        )
                    for s2 in range(NS2):
                        def chain(hi=hi, hh=hh, s2=s2):
                            if hh not in state:
                                alloc(hh)
                            ot_sb, _ = state[hh]
                            po = ps_po.tile(
                                [E1, S2], F32, tag="po", name=f"po{hh}_{s2}"
                            )
                            for tcj in range(SC):
                                nc.tensor.matmul(
                                    po[:],
                                    V1[:, tcj, hh, :],
                                    es2[:, tcj, hi, ds(s2 * S2, S2)],
                                    start=(tcj == 0),
                                    stop=(tcj == SC - 1),
                                )
                            nc.vector.tensor_copy(
                                out=ot_sb[:, ds(s2 * S2, S2)], in_=po[:]
                            )
                        def fin(hh=hh, s2=s2):
                            ot_sb, ob = state[hh]
                            for sc in range(s2 * sh, (s2 + 1) * sh):
                                tp = ps_po.tile(
                                    [P, P], F32, tag="po", name=f"tp{hh}_{sc}"
                                )
                                nc.tensor.transpose(
                                    tp[:, 0:E1],
                                    ot_sb[:, ts(sc, P)],
                                    id_f32[0:E1, 0:E1],
                                )
                                rec = small.tile([P, 1], F32, tag="rec")
                                nc.vector.reciprocal(out=rec[:], in_=tp[:, E:E1])
                                nc.vector.tensor_scalar_mul(
                                    ob[:, sc, :], tp[:, 0:E], rec[:]
                                )
                            if s2 == NS2 - 1:
                                nc.sync.dma_start(
                                    out=out[hh].rearrange("sc p e -> p sc e"),
                                    in_=ob[:],
                                )
                        jobs.append(chain)
                        jobs.append(fin)
                return jobs

            H2 = H // 2
            qts = {}
            pending_o = []
            for pr in range(H2):
                wp = wps.pop(pr)
                if pr + 1 < H2:
                    wps[pr + 1] = load_wpair(pr + 1)

                if pr == 0:
                    # pair 0's projections run serially at startup; do the
                    # s-half-0 chains of BOTH q and k first so the first
                    # score tiles only wait on half of x / W
                    qt2 = qk.tile([P, S], st_dt, tag="qt2", name="qt2_0")
                    kt2 = qk.tile([P, S], st_dt, tag="kt2", name="kt2_0")
                    qts[0] = (qt2, kt2)
                    jobs0 = emit_qkproj(0, wp, qt2, kt2)
                    for i in (0, NS2, 1, NS2 + 1) if NS2 > 1 else (0, 1):
                        jobs0[i]()
                qt2, kt2 = qts.pop(pr)

                # fill queue for this S phase
                fill = list(pending_o)
                pending_o = []
                if pr + 1 < H2:
                    nqt = qk.tile([P, S], st_dt, tag="qt2", name=f"qt2_{pr+1}")
                    nkt = qk.tile([P, S], st_dt, tag="kt2", name=f"kt2_{pr+1}")
                    qts[pr + 1] = (nqt, nkt)
                    fill += emit_qkproj(pr + 1, wps[pr + 1], nqt, nkt)
                if pr < NHE:
                    fill += vproj_jobs(pr)

                # scores + exp. Both heads' tiles for one (tcj, s2) share ONE
                # psum tile (adjacent banks): the matmuls use disjoint PE row
                # strips and issue back-to-back so the hardware co-executes
                # them; the single exp frees both banks simultaneously.
                es2 = es_pool.tile([P, SC, 2, S], es_dt, tag="es", name=f"es{pr}")

                # (tcj, s2) emission order: pair 0 starts with the tiles that
                # only need the s-half-0 projections; the last pair goes
                # s2-major so its own s2=0 O chains can run inside the phase.
                sh = SC // NS2
                if pr == H2 - 1 and NS2 > 1:
                    order = [(t, s) for s in range(NS2) for t in range(SC)]
                elif pr == 0 and NS2 > 1:
                    order = (
                        [(t, 0) for t in range(sh)]
                        + [(t, 1) for t in range(sh)]
                        + [(t, s) for t in range(sh, SC) for s in range(NS2)]
                    )
                else:
                    order = [(t, s) for t in range(SC) for s in range(NS2)]

                last_o = o_jobs(pr, es2) if pr == H2 - 1 else None
                if last_o is not None and NS2 > 1:
                    # s2=0 chains + finalizes of the last pair join the fill
                    # (they land in the second, s2=1 half of the phase)
                    fill = fill + [last_o[i] for i in (0, 1, 4, 5)]

                done = 0
                for k, (tcj, s2) in enumerate(order):
                    ps2 = ps_s.tile(
                        [P, 2, S2], F32, tag="s", name=f"ps{pr}_{tcj}_{s2}"
                    )
                    for hi in range(2):
                        nc.tensor.matmul(
                            ps2[:, hi, :],
                            kt2[ds(hi * E, E), ts(tcj, P)],
                            qt2[ds(hi * E, E), ds(s2 * S2, S2)],
                        )
                    nc.scalar.activation(
                        out=es2[:, tcj, :, ds(s2 * S2, S2)],
                        in_=ps2[:],
                        func=mybir.ActivationFunctionType.Exp,
                        scale=scale,
                    )
                    want = (k + 1) * len(fill) // len(order)
                    while done < want:
                        fill[done]()
                        done += 1

                if last_o is not None:
                    # both heads' s2=1 chains first, then their finalizes
                    rest = (2, 6, 3, 7) if NS2 > 1 else range(len(last_o))
                    for i in rest:
                        last_o[i]()
                    pending_o = []
                else:
                    pending_o = o_jobs(pr, es2)
            for job in pending_o:
                job()
    nc.finalize()
    return nc


_NC_CACHE = {}


def _get_nc(key=("bf16",)):
    if key not in _NC_CACHE:
        if key[0] == "bf16":
            _NC_CACHE[key] = build_nc(**FULL, st_dt=BF16)
        elif key[0] == "f32r":
            _NC_CACHE[key] = build_nc(**FULL, st_dt=F32R)
        elif key[0] == "hybrid":
            _NC_CACHE[key] = build_nc(**FULL, st_dt=F32R, es_dt=BF16)
        else:
            _NC_CACHE[key] = build_nc(**FULL, st_dt=F32)
    return _NC_CACHE[key]


# "bf16": fastest (~264 us, absmax-rel err ~1.2e-2)
# "hybrid": f32r scores + bf16 A@V (~285 us, ~2.7e-3)
# "f32r": max accuracy (~358 us, ~4.8e-4)
DEFAULT_VARIANT = os.environ.get("ATTN_VARIANT", "bf16")


def run_on_hw(x, WQ, WK, WV, variant=None, trace=False):
    from concourse.bass_utils import run_bass_kernel_spmd

    if variant is None:
        variant = DEFAULT_VARIANT

    nc = _get_nc((variant,))
    B = x.shape[0]
    assert B == N_CORES
    in_maps = [
        {
            "x": np.ascontiguousarray(x[b], dtype=np.float32),
            "WQ": np.ascontiguousarray(WQ, dtype=np.float32),
            "WK": np.ascontiguousarray(WK, dtype=np.float32),
            "WV": np.ascontiguousarray(WV, dtype=np.float32),
        }
        for b in range(B)
    ]
    res = run_bass_kernel_spmd(nc, in_maps, list(range(N_CORES)), trace=trace)
    outs = np.stack(
        [np.asarray(res.results[b]["out"]).reshape(-1) for b in range(B)], axis=0
    )
    return outs.astype(np.float32, copy=False), res


def kernel(x, WQ, WK, WV):
    outs, _ = run_on_hw(
        np.asarray(x), np.asarray(WQ), np.asarray(WK), np.asarray(WV)
    )
    return outs
